# revision 3
# baseline (speedup 1.0000x reference)
"""Trainium2 Bass kernel: 5-point Jacobi stencil with Dirichlet boundary.

out[b,0,i,j] = 0.25*(v[i-1,j]+v[i+1,j]+v[i,j-1]+v[i,j+1]) + cof*f[i,j]  (interior)
out boundary = 0, where v = u with boundary forced to 0, cof = -(1/1023)^2/4.

Sharding: data-parallel over batch, 2 images per core on 8 cores.

Per-core layout: image [1024,1024] -> SBUF tile [128 partitions, 8*1024],
partition p holds rows 8p..8p+7 (contiguous 32KB DMA lines). All stencil taps
are then same-partition free-dim shifts (+-1 horizontal, +-1024 vertical),
except the up-tap of row 8p (from partition p-1) and down-tap of row 8p+7
(from partition p+1), which are materialized once per image as halo tiles via
partition-shifted SBUF->SBUF DMA copies.

Engines: DVE does t1 = l+r, combine and final fused scale-add
(scalar_tensor_tensor); Pool (GPSIMD) does t2 = u+d; ACT does fcof = cof*f.
No TensorEngine (fp32r matmul truncates the streaming operand; fp32 matmul
too slow at 4 cyc/row).
"""
import numpy as np
import concourse.bacc as bacc
import concourse.bass as bass
import concourse.mybir as mybir
from concourse.tile import TileContext
from concourse.bass_utils import run_bass_kernel_spmd

N_CORES = 8
B_FULL = 16
H = 1024
W = 1024
IMGS = B_FULL // N_CORES  # images per core
P = 128                   # partitions
RPP = H // P              # rows per partition = 8
FREE = RPP * W            # 8192
PAD = 1
CHUNK_R = 2               # r-rows per compute chunk
CHUNK = CHUNK_R * W       # 2048
NCHUNK = RPP // CHUNK_R   # 4
COF = float(np.float32(-((1.0 / 1023.0) ** 2) / 4.0))
F32 = mybir.dt.float32

_cache = {}


def _build(trace_scopes=False):
    nc = bacc.Bacc("TRN2", target_bir_lowering=False)
    u_d = nc.dram_tensor("u", [IMGS, 1, H, W], F32, kind="ExternalInput")
    f_d = nc.dram_tensor("f", [IMGS, 1, H, W], F32, kind="ExternalInput")
    o_d = nc.dram_tensor("out", [IMGS, 1, H, W], F32, kind="ExternalOutput")

    add = mybir.AluOpType.add
    mult = mybir.AluOpType.mult

    with TileContext(nc) as tc:
        with (
            tc.tile_pool(name="upool", bufs=2) as upool,
            tc.tile_pool(name="halopool", bufs=2) as halopool,
            tc.tile_pool(name="fpool", bufs=2) as fpool,
            tc.tile_pool(name="t1pool", bufs=2) as t1pool,
            tc.tile_pool(name="t2pool", bufs=2) as t2pool,
            tc.tile_pool(name="opool", bufs=2) as opool,
            tc.tile_pool(name="zpool", bufs=1) as zpool,
        ):
            # zeros line used to zero partition-127 regions (engine ops may
            # not start at partition 127; DMA can write anywhere)
            zt = zpool.tile([1, W], F32, name="zt")
            nc.vector.memset(zt, 0.0)
            for b in range(IMGS):
                u_img = u_d[b, 0, :, :].rearrange("(p r) j -> p (r j)", r=RPP)
                f_img = f_d[b, 0, :, :].rearrange("(p r) j -> p (r j)", r=RPP)
                o_img = o_d[b, 0, :, :].rearrange("(p r) j -> p (r j)", r=RPP)

                ut = upool.tile([P, FREE + 2 * PAD], F32, name=f"ut{b}", tag="ut")
                nc.sync.dma_start(out=ut[:, PAD : PAD + FREE], in_=u_img)

                # v = u with boundary forced to 0 (+ pads zeroed)
                nc.vector.memset(ut[:, 0:PAD], 0.0)
                nc.vector.memset(ut[:, PAD + FREE : PAD + FREE + PAD], 0.0)
                nc.vector.memset(ut[0:1, PAD : PAD + W], 0.0)                     # row 0
                nc.sync.dma_start(out=ut[127:128, PAD + 7 * W : PAD + FREE], in_=zt)  # row 1023
                utv = ut[:, PAD : PAD + FREE].rearrange("p (r j) -> p r j", j=W)
                nc.vector.memset(utv[:, :, 0:1], 0.0)                             # col 0
                nc.vector.memset(utv[:, :, W - 1 : W], 0.0)                       # col 1023

                # halo tiles: Uh[p] = v[row 8p-1], Dh[p] = v[row 8p+8]
                uh = halopool.tile([P, W], F32, name=f"uh{b}", tag="uh")
                dh = halopool.tile([P, W], F32, name=f"dh{b}", tag="dh")
                nc.gpsimd.memset(uh[0:1, :], 0.0)
                nc.sync.dma_start(out=dh[127:128, :], in_=zt)
                nc.sync.dma_start(out=uh[1:128, :], in_=ut[0:127, PAD + 7 * W : PAD + FREE])
                nc.sync.dma_start(out=dh[0:127, :], in_=ut[1:128, PAD : PAD + W])

                for h in range(NCHUNK):
                    r0 = h * CHUNK_R
                    base = PAD + r0 * W
                    fc = fpool.tile([P, CHUNK], F32, name=f"fc{b}_{h}", tag="fc")
                    nc.sync.dma_start(
                        out=fc, in_=f_img[:, r0 * W : r0 * W + CHUNK]
                    )
                    t1 = t1pool.tile([P, CHUNK], F32, name=f"t1_{b}_{h}", tag="t1")
                    t2 = t2pool.tile([P, CHUNK], F32, name=f"t2_{b}_{h}", tag="t2")
                    oc = opool.tile([P, CHUNK], F32, name=f"oc{b}_{h}", tag="oc")

                    # t1 = left + right taps (free-dim +-1)
                    nc.vector.tensor_add(
                        out=t1,
                        in0=ut[:, base - 1 : base - 1 + CHUNK],
                        in1=ut[:, base + 1 : base + 1 + CHUNK],
                    )
                    # t2 = up + down taps (free-dim +-1024, halos at r=0 / r=7)
                    if h == 0:
                        nc.gpsimd.tensor_add(
                            out=t2[:, 0:W], in0=uh, in1=ut[:, PAD + W : PAD + 2 * W]
                        )
                        nc.gpsimd.tensor_add(
                            out=t2[:, W:CHUNK],
                            in0=ut[:, PAD : PAD + W],
                            in1=ut[:, PAD + 2 * W : PAD + 3 * W],
                        )
                    elif h == NCHUNK - 1:
                        nc.gpsimd.tensor_add(
                            out=t2[:, 0:W],
                            in0=ut[:, PAD + 5 * W : PAD + 6 * W],
                            in1=ut[:, PAD + 7 * W : PAD + FREE],
                        )
                        nc.gpsimd.tensor_add(
                            out=t2[:, W:CHUNK],
                            in0=ut[:, PAD + 6 * W : PAD + 7 * W],
                            in1=dh,
                        )
                    else:
                        nc.gpsimd.tensor_add(
                            out=t2,
                            in0=ut[:, base - W : base - W + CHUNK],
                            in1=ut[:, base + W : base + W + CHUNK],
                        )

                    # s = t1 + t2  (in-place into t1); split between DVE/Pool
                    if h in (1, 2):
                        nc.gpsimd.tensor_add(out=t1, in0=t1, in1=t2)
                    else:
                        nc.vector.tensor_add(out=t1, in0=t1, in1=t2)

                    # fcof = cof * f (ACT, in-place)
                    nc.scalar.mul(fc, fc, COF)

                    # out = 0.25*s + fcof (fused on DVE)
                    nc.vector.scalar_tensor_tensor(
                        out=oc, in0=t1, scalar=0.25, in1=fc, op0=mult, op1=add
                    )

                    # zero output boundary inside this chunk
                    ocv = oc.rearrange("p (r j) -> p r j", j=W)
                    nc.vector.memset(ocv[:, :, 0:1], 0.0)
                    nc.vector.memset(ocv[:, :, W - 1 : W], 0.0)
                    if h == 0:
                        nc.vector.memset(oc[0:1, 0:W], 0.0)          # row 0
                    if h == NCHUNK - 1:
                        nc.sync.dma_start(out=oc[127:128, W:CHUNK], in_=zt)  # row 1023

                    nc.sync.dma_start(
                        out=o_img[:, r0 * W : r0 * W + CHUNK], in_=oc
                    )
    nc.finalize()
    return nc


def _get_nc():
    if "nc" not in _cache:
        _cache["nc"] = _build()
    return _cache["nc"]


def _run(u, f, trace=False):
    u = np.ascontiguousarray(np.asarray(u, dtype=np.float32))
    f = np.ascontiguousarray(np.asarray(f, dtype=np.float32))
    nc = _get_nc()
    in_maps = [
        {"u": u[i * IMGS : (i + 1) * IMGS], "f": f[i * IMGS : (i + 1) * IMGS]}
        for i in range(N_CORES)
    ]
    res = run_bass_kernel_spmd(nc, in_maps, core_ids=list(range(N_CORES)), trace=trace)
    out = np.concatenate([r["out"] for r in res.results], axis=0)
    return out, res


def kernel(u, f, weight=None):
    out, _ = _run(u, f)
    return out


# revision 4
# speedup vs baseline: 80.0561x; 80.0561x over previous
"""Trainium2 Bass kernel: 5-point Jacobi stencil with Dirichlet boundary.

out[b,0,i,j] = 0.25*(v[i-1,j]+v[i+1,j]+v[i,j-1]+v[i,j+1]) + cof*f[i,j]  (interior)
out boundary = 0, where v = u with boundary forced to 0, cof = -(1/1023)^2/4.

Sharding: data-parallel over batch, 2 images per core on 8 cores.

Per-core layout: image [1024,1024] -> SBUF tile [128 partitions, 8*1024],
partition p holds rows 8p..8p+7 (contiguous 32KB DMA lines). All stencil taps
are then same-partition free-dim shifts (+-1 horizontal, +-1024 vertical),
except the up-tap of row 8p (from partition p-1) and down-tap of row 8p+7
(from partition p+1), which are materialized once per image as halo tiles via
partition-shifted SBUF->SBUF DMA copies.

Engines: DVE does t1 = l+r, combine and final fused scale-add
(scalar_tensor_tensor); Pool (GPSIMD) does t2 = u+d; ACT does fcof = cof*f.
No TensorEngine (fp32r matmul truncates the streaming operand; fp32 matmul
too slow at 4 cyc/row).
"""
import numpy as np
import concourse.bacc as bacc
import concourse.bass as bass
import concourse.mybir as mybir
from concourse.tile import TileContext
from concourse.bass_utils import run_bass_kernel_spmd

N_CORES = 8
B_FULL = 16
H = 1024
W = 1024
IMGS = B_FULL // N_CORES  # images per core
P = 128                   # partitions
RPP = H // P              # rows per partition = 8
FREE = RPP * W            # 8192
PAD = 1
CHUNK_R = 2               # r-rows per compute chunk
CHUNK = CHUNK_R * W       # 2048
NCHUNK = RPP // CHUNK_R   # 4
COF = float(np.float32(-((1.0 / 1023.0) ** 2) / 4.0))
F32 = mybir.dt.float32

_cache = {}


def _build(repeat=1):
    nc = bacc.Bacc("TRN2", target_bir_lowering=False)
    u_d = nc.dram_tensor("u", [IMGS, 1, H, W], F32, kind="ExternalInput")
    f_d = nc.dram_tensor("f", [IMGS, 1, H, W], F32, kind="ExternalInput")
    o_d = nc.dram_tensor("out", [IMGS, 1, H, W], F32, kind="ExternalOutput")

    add = mybir.AluOpType.add
    mult = mybir.AluOpType.mult

    with TileContext(nc) as tc:
        with (
            tc.tile_pool(name="upool", bufs=2) as upool,
            tc.tile_pool(name="halopool", bufs=2) as halopool,
            tc.tile_pool(name="fpool", bufs=2) as fpool,
            tc.tile_pool(name="t1pool", bufs=2) as t1pool,
            tc.tile_pool(name="t2pool", bufs=2) as t2pool,
            tc.tile_pool(name="opool", bufs=2) as opool,
            tc.tile_pool(name="zpool", bufs=1) as zpool,
        ):
            # zeros line used to zero partition-127 regions (engine ops may
            # not start at partition 127; DMA can write anywhere)
            zt = zpool.tile([1, W], F32, name="zt")
            nc.vector.memset(zt, 0.0)
            for b in range(IMGS * repeat):
                b = b % IMGS
                u_img = u_d[b, 0, :, :].rearrange("(p r) j -> p (r j)", r=RPP)
                f_img = f_d[b, 0, :, :].rearrange("(p r) j -> p (r j)", r=RPP)
                o_img = o_d[b, 0, :, :].rearrange("(p r) j -> p (r j)", r=RPP)

                ut = upool.tile([P, FREE + 2 * PAD], F32, name=f"ut{b}", tag="ut")
                nc.sync.dma_start(out=ut[:, PAD : PAD + FREE], in_=u_img)

                # v = u with boundary forced to 0 (+ pads zeroed)
                nc.vector.memset(ut[:, 0:PAD], 0.0)
                nc.vector.memset(ut[:, PAD + FREE : PAD + FREE + PAD], 0.0)
                nc.vector.memset(ut[0:1, PAD : PAD + W], 0.0)                     # row 0
                nc.sync.dma_start(out=ut[127:128, PAD + 7 * W : PAD + FREE], in_=zt)  # row 1023
                utv = ut[:, PAD : PAD + FREE].rearrange("p (r j) -> p r j", j=W)
                nc.vector.memset(utv[:, :, 0:1], 0.0)                             # col 0
                nc.vector.memset(utv[:, :, W - 1 : W], 0.0)                       # col 1023

                # halo tiles: Uh[p] = v[row 8p-1], Dh[p] = v[row 8p+8]
                uh = halopool.tile([P, W], F32, name=f"uh{b}", tag="uh")
                dh = halopool.tile([P, W], F32, name=f"dh{b}", tag="dh")
                nc.gpsimd.memset(uh[0:1, :], 0.0)
                nc.sync.dma_start(out=dh[127:128, :], in_=zt)
                nc.sync.dma_start(out=uh[1:128, :], in_=ut[0:127, PAD + 7 * W : PAD + FREE])
                nc.sync.dma_start(out=dh[0:127, :], in_=ut[1:128, PAD : PAD + W])

                for h in range(NCHUNK):
                    r0 = h * CHUNK_R
                    base = PAD + r0 * W
                    fc = fpool.tile([P, CHUNK], F32, name=f"fc{b}_{h}", tag="fc")
                    nc.sync.dma_start(
                        out=fc, in_=f_img[:, r0 * W : r0 * W + CHUNK]
                    )
                    t1 = t1pool.tile([P, CHUNK], F32, name=f"t1_{b}_{h}", tag="t1")
                    t2 = t2pool.tile([P, CHUNK], F32, name=f"t2_{b}_{h}", tag="t2")
                    oc = opool.tile([P, CHUNK], F32, name=f"oc{b}_{h}", tag="oc")

                    # t1 = left + right taps (free-dim +-1)
                    nc.vector.tensor_add(
                        out=t1,
                        in0=ut[:, base - 1 : base - 1 + CHUNK],
                        in1=ut[:, base + 1 : base + 1 + CHUNK],
                    )
                    # t2 = up + down taps (free-dim +-1024, halos at r=0 / r=7)
                    if h == 0:
                        nc.gpsimd.tensor_add(
                            out=t2[:, 0:W], in0=uh, in1=ut[:, PAD + W : PAD + 2 * W]
                        )
                        nc.gpsimd.tensor_add(
                            out=t2[:, W:CHUNK],
                            in0=ut[:, PAD : PAD + W],
                            in1=ut[:, PAD + 2 * W : PAD + 3 * W],
                        )
                    elif h == NCHUNK - 1:
                        nc.gpsimd.tensor_add(
                            out=t2[:, 0:W],
                            in0=ut[:, PAD + 5 * W : PAD + 6 * W],
                            in1=ut[:, PAD + 7 * W : PAD + FREE],
                        )
                        nc.gpsimd.tensor_add(
                            out=t2[:, W:CHUNK],
                            in0=ut[:, PAD + 6 * W : PAD + 7 * W],
                            in1=dh,
                        )
                    else:
                        nc.gpsimd.tensor_add(
                            out=t2,
                            in0=ut[:, base - W : base - W + CHUNK],
                            in1=ut[:, base + W : base + W + CHUNK],
                        )

                    # s = t1 + t2  (in-place into t1); split between DVE/Pool
                    if h in (1, 2):
                        nc.gpsimd.tensor_add(out=t1, in0=t1, in1=t2)
                    else:
                        nc.vector.tensor_add(out=t1, in0=t1, in1=t2)

                    # fcof = cof * f (ACT, in-place)
                    nc.scalar.mul(fc, fc, COF)

                    # out = 0.25*s + fcof (fused on DVE)
                    nc.vector.scalar_tensor_tensor(
                        out=oc, in0=t1, scalar=0.25, in1=fc, op0=mult, op1=add
                    )

                    # zero output boundary inside this chunk
                    ocv = oc.rearrange("p (r j) -> p r j", j=W)
                    nc.vector.memset(ocv[:, :, 0:1], 0.0)
                    nc.vector.memset(ocv[:, :, W - 1 : W], 0.0)
                    if h == 0:
                        nc.vector.memset(oc[0:1, 0:W], 0.0)          # row 0
                    if h == NCHUNK - 1:
                        nc.sync.dma_start(out=oc[127:128, W:CHUNK], in_=zt)  # row 1023

                    nc.sync.dma_start(
                        out=o_img[:, r0 * W : r0 * W + CHUNK], in_=oc
                    )
    nc.finalize()
    return nc


def _get_nc(repeat=1):
    if repeat not in _cache:
        _cache[repeat] = _build(repeat)
    return _cache[repeat]


def _run(u, f, trace=False):
    u = np.ascontiguousarray(np.asarray(u, dtype=np.float32))
    f = np.ascontiguousarray(np.asarray(f, dtype=np.float32))
    nc = _get_nc()
    in_maps = [
        {"u": u[i * IMGS : (i + 1) * IMGS], "f": f[i * IMGS : (i + 1) * IMGS]}
        for i in range(N_CORES)
    ]
    res = run_bass_kernel_spmd(nc, in_maps, core_ids=list(range(N_CORES)), trace=trace)
    out = np.concatenate([r["out"] for r in res.results], axis=0)
    return out, res


def kernel(u, f, weight=None):
    out, _ = _run(u, f)
    return out
